# revision 1
# baseline (speedup 1.0000x reference)
"""Trainium2 Bass kernel for nn_DKSTE_85315230367936 (embedding_lookup).

Math (per batch element b, dim d, with K=2 planes):
    x = sign(rel[b,d,0]); y = sign(rel[b,d,1]); a = sign(alpha[b,d])
    s = (x+y)/2 ; dd = (x-y)/2
    term = h0*(s*t0 - dd*a*t1) + h1*(dd*t0 + s*a*t1)
         = s*(h0*t0 + a*h1*t1) + dd*(h1*t0 - a*h0*t1)   [identical algebra]
    out[b] = sqrt(sum_d term^2)

Strategy: pure data parallelism over the batch (1024 elements/core); the
entity table is replicated into every core's HBM (host-side upload) as a
single [200000, 1024] fp16 table whose rows are [k=0 plane | k=1 plane].
Per core:
  1. precompute sign tables on device: s2=sign(x)+sign(y), d2=sign(x)-sign(y),
     a=sign(alpha) packed as one fp16 [500, 1536] DRAM table (the /2 of s,d is
     folded into the final sqrt scale).  ScalarE computes the signs with
     deinterleaved (stride-2) reads so the VectorE combines run contiguous
     fp16 at 2x rate.
  2. per 128-element tile: three gpsimd indirect-DMA row gathers ([128,1]
     int32 offsets — the only offset layout the ucode supports) for head
     rows, tail rows, and sign-table rows.
  3. VectorE fp16 elementwise chain (11 tensor_tensor ops; the four
     entity-only products can overlap the sign-table precompute), ScalarE
     Square+accumulate reduction over d, final sqrt(0.25 * acc).
Output [128, 8] f32 per core; host inverse-permutes to [8192].
"""

import sys

for _p in ("/opt/trn_rl_repo",):
    if _p not in sys.path:
        sys.path.insert(0, _p)

import numpy as np

import concourse.bass as bass
import concourse.bacc as bacc
import concourse.tile as tile
from concourse import mybir
from concourse.bass_utils import run_bass_kernel_spmd

NENTITY, NRELATION, EMB_DIM, K = 200000, 500, 512, 2
BATCH = 8192
NCORES = 8
B_LOC = BATCH // NCORES            # 1024 batch elements per core
NT = B_LOC // 128                  # 8 tiles of 128 per core
CDT = mybir.dt.float16             # compute dtype on device
NP_CDT = np.float16

F32 = mybir.dt.float32
I16 = mybir.dt.int16
I32 = mybir.dt.int32
AF = mybir.ActivationFunctionType
ALU = mybir.AluOpType

# relation/alpha tables flattened across 125 partitions (4 relation rows per
# partition so the sign-table DMA out reshapes cleanly to [125, 4, 512])
REL_P = 125
REL_FREE = NRELATION * EMB_DIM * K // REL_P  # 4096
AL_FREE = NRELATION * EMB_DIM // REL_P       # 2048
SDA_W = 3 * EMB_DIM                          # 1536


def build_program():
    nc = bacc.Bacc("TRN2", target_bir_lowering=False, debug=False,
                   num_swdge_queues=4)

    ea = nc.declare_dram_parameter("ea", [NENTITY, 2 * EMB_DIM], CDT, isOutput=False)
    relf = nc.declare_dram_parameter("relf", [REL_P, REL_FREE], CDT, isOutput=False)
    alphaf = nc.declare_dram_parameter("alphaf", [REL_P, AL_FREE], CDT, isOutput=False)
    htidx = nc.declare_dram_parameter("htidx", [128, 2 * NT], I32, isOutput=False)
    relidx = nc.declare_dram_parameter("relidx", [128, B_LOC // 16], I16, isOutput=False)
    out = nc.declare_dram_parameter("out", [128, NT], F32, isOutput=True)

    with tile.TileContext(nc) as tc:
        with (
            tc.tile_pool(name="dram", bufs=1, space="DRAM") as dramp,
            tc.tile_pool(name="idx", bufs=1) as idxp,
            tc.tile_pool(name="prep", bufs=1) as prep,
            tc.tile_pool(name="gat", bufs=4) as gat,
            tc.tile_pool(name="wrk", bufs=3) as wrk,
            tc.tile_pool(name="outp", bufs=1) as outp,
        ):
            # internal DRAM: per-relation [s2 | d2 | a] rows of 3*512 fp16
            sda = dramp.tile([NRELATION, SDA_W], CDT)

            # ---- index upload -------------------------------------------
            ht_t = idxp.tile([128, 2 * NT], I32)
            nc.sync.dma_start(out=ht_t[:], in_=htidx[:])
            rel_t = idxp.tile([128, B_LOC // 16], I16)
            nc.sync.dma_start(out=rel_t[:], in_=relidx[:])

            # ---- sign-table precompute ----------------------------------
            relsb = prep.tile([REL_P, REL_FREE], CDT)
            nc.sync.dma_start(out=relsb[:], in_=relf[:])
            alsb = prep.tile([REL_P, AL_FREE], CDT)
            nc.scalar.dma_start(out=alsb[:], in_=alphaf[:])
            sx = prep.tile([REL_P, REL_FREE // 2], CDT)
            nc.scalar.activation(sx[:], relsb[:, 0::2], AF.Sign)
            sy = prep.tile([REL_P, REL_FREE // 2], CDT)
            nc.scalar.activation(sy[:], relsb[:, 1::2], AF.Sign)
            # one SBUF image of the sda table (4 relation rows per partition,
            # row-blocked [s2 | d2 | a]) so the DRAM write is ONE contiguous DMA
            sda_sb = prep.tile([REL_P, 4 * SDA_W], CDT)
            sda_sbv = sda_sb[:].rearrange("p (r c d) -> p r c d", c=3, d=EMB_DIM)
            sx3 = sx[:].rearrange("p (r d) -> p r d", d=EMB_DIM)
            sy3 = sy[:].rearrange("p (r d) -> p r d", d=EMB_DIM)
            nc.vector.tensor_tensor(
                out=sda_sbv[:, :, 0, :], in0=sx3, in1=sy3, op=ALU.add
            )
            nc.vector.tensor_tensor(
                out=sda_sbv[:, :, 1, :], in0=sx3, in1=sy3, op=ALU.subtract
            )
            nc.scalar.activation(
                sda_sbv[:, :, 2, :],
                alsb[:].rearrange("p (r d) -> p r d", d=EMB_DIM),
                AF.Sign,
            )
            nc.sync.dma_start(
                out=sda[:].rearrange("(p r) w -> p (r w)", r=4), in_=sda_sb[:]
            )

            # preload the Sqrt LUT during the precompute window so the final
            # sqrt doesn't pay the ACT table swap on the critical tail
            sq_dummy = outp.tile([128, 1], F32)
            nc.gpsimd.memset(sq_dummy[:], 1.0)
            nc.scalar.activation(sq_dummy[:], sq_dummy[:], AF.Sqrt)

            # ---- main loop ----------------------------------------------
            scores = outp.tile([128, NT], F32)
            junk = outp.tile([128, EMB_DIM], CDT)

            def tt(tag, in0, in1, op):
                t = wrk.tile([128, EMB_DIM], CDT, tag=tag)
                nc.vector.tensor_tensor(out=t[:], in0=in0, in1=in1, op=op)
                return t

            qn = [0]

            def igather(out_ap, in_ap, off_ap):
                # round-robin the 4 SWDGE queues so gather payloads spread
                # across more SDMA engines and desc-gen isn't ring-blocked
                inst = nc.gpsimd.indirect_dma_start(
                    out=out_ap, out_offset=None, in_=in_ap,
                    in_offset=bass.IndirectOffsetOnAxis(ap=off_ap, axis=0),
                )
                q = qn[0] % 4
                qn[0] += 1
                if q:
                    inst.ins.queue = f"qPoolDynamic{q}"
                return inst

            for t in range(NT):
                gh = gat.tile([128, 2 * EMB_DIM], CDT, tag="gh")
                igather(gh[:], ea[:], ht_t[:, 2 * t : 2 * t + 1])
                gt = gat.tile([128, 2 * EMB_DIM], CDT, tag="gt")
                igather(gt[:], ea[:], ht_t[:, 2 * t + 1 : 2 * t + 2])
                gs = gat.tile([128, 1, SDA_W], CDT, tag="gs")
                nc.gpsimd.dma_gather(
                    gs[:], sda[:], rel_t[:, 8 * t : 8 * (t + 1)], 128, 128, SDA_W
                )

                h0 = gh[:, 0:EMB_DIM]
                h1 = gh[:, EMB_DIM : 2 * EMB_DIM]
                t0 = gt[:, 0:EMB_DIM]
                t1 = gt[:, EMB_DIM : 2 * EMB_DIM]
                s2v = gs[:, 0, 0:EMB_DIM]
                d2v = gs[:, 0, EMB_DIM : 2 * EMB_DIM]
                av = gs[:, 0, 2 * EMB_DIM : 3 * EMB_DIM]

                # entity-only products first: these can run while the
                # sign-table precompute is still in flight
                m1 = tt("m1", h0, t0, ALU.mult)
                m2 = tt("m2", h1, t1, ALU.mult)
                m4 = tt("m4", h1, t0, ALU.mult)
                m5 = tt("m5", h0, t1, ALU.mult)
                m3 = tt("m3", av, m2[:], ALU.mult)
                A = tt("A", m1[:], m3[:], ALU.add)
                m6 = tt("m6", av, m5[:], ALU.mult)
                B = tt("B", m4[:], m6[:], ALU.subtract)
                u = tt("u", s2v, A[:], ALU.mult)
                w = tt("w", d2v, B[:], ALU.mult)
                term = tt("term", u[:], w[:], ALU.add)

                nc.scalar.activation(
                    junk[:],
                    term[:],
                    AF.Square,
                    accum_out=scores[:, t : t + 1],
                )

            res = outp.tile([128, NT], F32)
            # score = sqrt(sum(term^2)) = sqrt(0.25 * sum((2*term)^2))
            nc.scalar.activation(res[:], scores[:], AF.Sqrt, scale=0.25)
            nc.sync.dma_start(out=out[:], in_=res[:])

    nc.compile()
    return nc


_NC_CACHE = None


def _get_program():
    global _NC_CACHE
    if _NC_CACHE is None:
        _NC_CACHE = build_program()
    return _NC_CACHE


def make_in_maps(head_idx, relation_idx, tail_idx, entity_embedding,
                 relation_embedding, alpha_embedding):
    """Host-side sharding: slice batch 1024/core, replicate tables."""
    head_idx = np.asarray(head_idx).astype(np.int32)
    relation_idx = np.asarray(relation_idx).astype(np.int32)
    tail_idx = np.asarray(tail_idx).astype(np.int32)
    ent = np.asarray(entity_embedding)
    rel = np.asarray(relation_embedding)
    alp = np.asarray(alpha_embedding)

    # ea row r = [E[r,:,0,0] | E[r,:,0,1]]
    ea = np.ascontiguousarray(
        ent[:, :, 0, :].transpose(0, 2, 1).reshape(NENTITY, 2 * EMB_DIM)
    ).astype(NP_CDT)
    relf = rel.astype(NP_CDT).reshape(REL_P, REL_FREE)
    alphaf = alp.astype(NP_CDT).reshape(REL_P, AL_FREE)

    in_maps = []
    for c in range(NCORES):
        lo = c * B_LOC
        h = head_idx[lo : lo + B_LOC]
        tl = tail_idx[lo : lo + B_LOC]
        r = relation_idx[lo : lo + B_LOC]
        # htidx[p, 2t] = head of elem t*128+p ; [p, 2t+1] = tail
        htp = np.empty((128, 2 * NT), np.int32)
        for t in range(NT):
            htp[:, 2 * t] = h[128 * t : 128 * (t + 1)]
            htp[:, 2 * t + 1] = tl[128 * t : 128 * (t + 1)]
        # dma_gather idx wrap: idx i lives at [i % 16, i // 16], replicated
        # across the 8 16-partition groups
        rwrap = np.zeros((16, B_LOC // 16), np.int16)
        ii = np.arange(B_LOC)
        rwrap[ii % 16, ii // 16] = r.astype(np.int16)
        rlp = np.tile(rwrap, (8, 1))
        in_maps.append(
            {
                "ea": ea,
                "relf": relf,
                "alphaf": alphaf,
                "htidx": htp,
                "relidx": rlp,
            }
        )
    return in_maps


def unshard_out(results):
    """results: list of per-core dicts with 'out' [128, NT] f32."""
    full = np.empty(BATCH, np.float32)
    for c in range(NCORES):
        o = np.asarray(results[c]["out"])          # [128, NT], col = t
        # elem 128t + p  <-  o[p, t]
        full[c * B_LOC : (c + 1) * B_LOC] = o.T.ravel()
    return full


def kernel(head_idx, relation_idx, tail_idx, entity_embedding,
           relation_embedding, alpha_embedding):
    nc = _get_program()
    in_maps = make_in_maps(head_idx, relation_idx, tail_idx, entity_embedding,
                           relation_embedding, alpha_embedding)
    res = run_bass_kernel_spmd(nc, in_maps, list(range(NCORES)))
    return unshard_out(res.results)



# revision 4
# speedup vs baseline: 1.2236x; 1.2236x over previous
"""Trainium2 Bass kernel for nn_DKSTE_85315230367936 (embedding_lookup).

Math (per batch element b, dim d, K=2 planes):
    x = sign(rel[b,d,0]); y = sign(rel[b,d,1]); a = sign(alpha[b,d])
    s = (x+y)/2 ; dd = (x-y)/2   (exactly one of s, dd is nonzero)
    term = h0*(s*t0 - dd*a*t1) + h1*(dd*t0 + s*a*t1)
    out[b] = sqrt(sum_d term^2)

Only |term| matters (squared).  With m = [sign(x)==sign(y)] and
phi = a*sign(x*y):
    m=1: term^2 = u^2,  u = h0*t0 + h1*(phi*t1)
    m=0: term^2 = v^2,  v = h1*t0 + h0*(phi*t1)
so z = select(m, u, v), out = sqrt(sum_d z^2); phi*t1 is shared.

Per-core (1024 elems, 8 tiles of 128):
  - entity table replicated per core: [200000, 1024] fp16, row [E1|E0]
    (k-planes swapped so the wide fused op below gets [h1|h0] directly).
  - device builds sda [500, 768] fp16-units: [phi(512 fp16) | m(512 int8)]
    from the uploaded (host-deinterleaved) relation x/y planes + alpha.
  - per tile, three [128,1]-offset indirect gathers (the only offset
    layout the SWDGE ucode supports; multi-column offsets fetch
    contiguous bursts instead): head row, tail row, sda row.  All 24
    calls are interleaved tile-by-tile so compute chases the gathers.
  - per tile DVE: u1,v1 into UV1=[u1|v1]; pt1=phi*t1;
    W=[h1|h0]*bcast(pt1)=[u2|v2]; UV=UV1+W=[u|v];
    copy_predicated(v-half, m, u-half) -> z; ACT square-accumulate.
"""

import sys

for _p in ("/opt/trn_rl_repo",):
    if _p not in sys.path:
        sys.path.insert(0, _p)

import numpy as np

import concourse.bass as bass
import concourse.bacc as bacc
import concourse.tile as tile
from concourse import mybir
from concourse.bass_utils import run_bass_kernel_spmd

NENTITY, NRELATION, EMB_DIM, K = 200000, 500, 512, 2
BATCH = 8192
NCORES = 8
B_LOC = BATCH // NCORES            # 1024 batch elements per core
NT = B_LOC // 128                  # 8 tiles of 128 per core
CDT = mybir.dt.float16
NP_CDT = np.float16

F32 = mybir.dt.float32
I8 = mybir.dt.int8
I16 = mybir.dt.int16
I32 = mybir.dt.int32
AF = mybir.ActivationFunctionType
ALU = mybir.AluOpType

REL_P = 125                                  # 4 relation rows per partition
RX_FREE = NRELATION * EMB_DIM // REL_P       # 2048
SDA_W = EMB_DIM + EMB_DIM // 2               # 768 fp16 units = 1536 B


def build_program():
    nc = bacc.Bacc("TRN2", target_bir_lowering=False, debug=False,
                   num_swdge_queues=4)

    ea = nc.declare_dram_parameter("ea", [NENTITY, 2 * EMB_DIM], CDT, isOutput=False)
    relx = nc.declare_dram_parameter("relx", [REL_P, RX_FREE], CDT, isOutput=False)
    rely = nc.declare_dram_parameter("rely", [REL_P, RX_FREE], CDT, isOutput=False)
    alphaf = nc.declare_dram_parameter("alphaf", [REL_P, RX_FREE], CDT, isOutput=False)
    htidx = nc.declare_dram_parameter("htidx", [128, 2 * NT], I32, isOutput=False)
    relidx = nc.declare_dram_parameter("relidx", [128, NT], I32, isOutput=False)
    out = nc.declare_dram_parameter("out", [128, NT], F32, isOutput=True)

    with tile.TileContext(nc) as tc:
        with (
            tc.tile_pool(name="dram", bufs=1, space="DRAM") as dramp,
            tc.tile_pool(name="idx", bufs=1) as idxp,
            tc.tile_pool(name="prep", bufs=1) as prep,
            tc.tile_pool(name="gat", bufs=4) as gat,
            tc.tile_pool(name="gsd", bufs=4) as gsdp,
            tc.tile_pool(name="wrk", bufs=2) as wrk,
            tc.tile_pool(name="outp", bufs=1) as outp,
        ):
            sda = dramp.tile([NRELATION, SDA_W], CDT)

            # ---- index + table uploads (HWDGE; first so gathers unblock)
            ht_t = idxp.tile([128, 2 * NT], I32)
            nc.sync.dma_start(out=ht_t[:], in_=htidx[:])
            rel_t = idxp.tile([128, NT], I32)
            nc.sync.dma_start(out=rel_t[:], in_=relidx[:])
            relxb = prep.tile([REL_P, RX_FREE], CDT)
            nc.sync.dma_start(out=relxb[:], in_=relx[:])
            relyb = prep.tile([REL_P, RX_FREE], CDT)
            nc.scalar.dma_start(out=relyb[:], in_=rely[:])
            alsb = prep.tile([REL_P, RX_FREE], CDT)
            nc.scalar.dma_start(out=alsb[:], in_=alphaf[:])

            # ---- sign-table precompute --------------------------------
            xy = prep.tile([REL_P, RX_FREE], CDT)
            nc.vector.tensor_tensor(out=xy[:], in0=relxb[:], in1=relyb[:],
                                    op=ALU.mult)
            sxy = prep.tile([REL_P, RX_FREE], CDT)
            nc.scalar.activation(sxy[:], xy[:], AF.Sign)
            asg = prep.tile([REL_P, RX_FREE], CDT)
            nc.scalar.activation(asg[:], alsb[:], AF.Sign)

            # sda image: 4 relation blocks of 768 per partition,
            # each block = [phi(512 fp16) | m(512 int8)]
            sda_sb = prep.tile([REL_P, 4 * SDA_W], CDT)
            sda_sbv = sda_sb[:].rearrange("p (r c) -> p r c", c=SDA_W)
            a3 = asg[:].rearrange("p (r d) -> p r d", d=EMB_DIM)
            s3 = sxy[:].rearrange("p (r d) -> p r d", d=EMB_DIM)
            xy3 = xy[:].rearrange("p (r d) -> p r d", d=EMB_DIM)
            nc.vector.tensor_tensor(
                out=sda_sbv[:, :, 0:EMB_DIM], in0=a3, in1=s3, op=ALU.mult
            )
            nc.vector.tensor_scalar(
                out=sda_sbv[:, :, EMB_DIM:SDA_W].bitcast(I8),
                in0=xy3, scalar1=0.0, scalar2=None, op0=ALU.is_gt,
            )
            nc.sync.dma_start(
                out=sda[:].rearrange("(p r) w -> p (r w)", r=4), in_=sda_sb[:]
            )

            # ---- main loop: 3 gathers per tile, compute chases --------
            scores = outp.tile([128, NT], F32)
            junk = outp.tile([128, EMB_DIM], CDT)
            qn = [0]

            def igather(out_ap, in_ap, off_ap):
                inst = nc.gpsimd.indirect_dma_start(
                    out=out_ap, out_offset=None, in_=in_ap,
                    in_offset=bass.IndirectOffsetOnAxis(ap=off_ap, axis=0),
                )
                q = qn[0] % 4
                qn[0] += 1
                if q:
                    inst.ins.queue = f"qPoolDynamic{q}"
                return inst

            for t in range(NT):
                ght = gat.tile([128, 2048], CDT, tag="ght")
                igather(ght[:, 0:1024], ea[:], ht_t[:, 2 * t : 2 * t + 1])
                igather(ght[:, 1024:2048], ea[:], ht_t[:, 2 * t + 1 : 2 * t + 2])
                gsd = gsdp.tile([128, SDA_W], CDT, tag="gsd")
                igather(gsd[:], sda[:], rel_t[:, t : t + 1])

                # entity row layout [E1|E0]: head slice = [h1|h0]
                h1 = ght[:, 0:512]
                h0 = ght[:, 512:1024]
                t1 = ght[:, 1024:1536]
                t0 = ght[:, 1536:2048]
                phi = gsd[:, 0:EMB_DIM]
                msk = gsd[:, EMB_DIM:SDA_W].bitcast(I8)

                uv1 = wrk.tile([128, 2, EMB_DIM], CDT, tag="uv1")
                nc.vector.tensor_tensor(out=uv1[:, 0, :], in0=h0, in1=t0,
                                        op=ALU.mult)
                nc.vector.tensor_tensor(out=uv1[:, 1, :], in0=h1, in1=t0,
                                        op=ALU.mult)
                pt1 = wrk.tile([128, EMB_DIM], CDT, tag="pt1")
                nc.vector.tensor_tensor(out=pt1[:], in0=phi, in1=t1,
                                        op=ALU.mult)
                w2 = wrk.tile([128, 2, EMB_DIM], CDT, tag="w2")
                pt1b = pt1[:].rearrange("p (o d) -> p o d", o=1).broadcast_to(
                    [128, 2, EMB_DIM]
                )
                nc.vector.tensor_tensor(
                    out=w2[:],
                    in0=ght[:, 0:1024].rearrange("p (o d) -> p o d", d=EMB_DIM),
                    in1=pt1b, op=ALU.mult,
                )
                uv = wrk.tile([128, 2, EMB_DIM], CDT, tag="uv")
                nc.vector.tensor_tensor(out=uv[:], in0=uv1[:], in1=w2[:],
                                        op=ALU.add)
                # uv = [u | v]; z = select(m, u, v) in place in the v half
                nc.vector.copy_predicated(uv[:, 1, :], msk, uv[:, 0, :])

                nc.scalar.activation(
                    junk[:], uv[:, 1, :], AF.Square,
                    accum_out=scores[:, t : t + 1],
                )

            res = outp.tile([128, NT], F32)
            nc.scalar.activation(res[:], scores[:], AF.Sqrt)
            nc.sync.dma_start(out=out[:], in_=res[:])

    nc.compile()
    return nc


_NC_CACHE = None


def _get_program():
    global _NC_CACHE
    if _NC_CACHE is None:
        _NC_CACHE = build_program()
    return _NC_CACHE


def make_in_maps(head_idx, relation_idx, tail_idx, entity_embedding,
                 relation_embedding, alpha_embedding):
    """Host-side sharding: slice batch 1024/core, replicate tables."""
    head_idx = np.asarray(head_idx).astype(np.int32)
    relation_idx = np.asarray(relation_idx).astype(np.int32)
    tail_idx = np.asarray(tail_idx).astype(np.int32)
    ent = np.asarray(entity_embedding)
    rel = np.asarray(relation_embedding)
    alp = np.asarray(alpha_embedding)

    # ea row r = [E[r,:,0,1] | E[r,:,0,0]]  (k planes swapped)
    ea = np.ascontiguousarray(
        ent[:, :, 0, ::-1].transpose(0, 2, 1).reshape(NENTITY, 2 * EMB_DIM)
    ).astype(NP_CDT)
    relx = np.ascontiguousarray(rel[:, :, 0]).astype(NP_CDT).reshape(REL_P, RX_FREE)
    rely = np.ascontiguousarray(rel[:, :, 1]).astype(NP_CDT).reshape(REL_P, RX_FREE)
    alphaf = alp.astype(NP_CDT).reshape(REL_P, RX_FREE)

    in_maps = []
    for c in range(NCORES):
        lo = c * B_LOC
        h = head_idx[lo : lo + B_LOC]
        tl = tail_idx[lo : lo + B_LOC]
        r = relation_idx[lo : lo + B_LOC]
        htp = np.empty((128, 2 * NT), np.int32)
        rlp = np.empty((128, NT), np.int32)
        for t in range(NT):
            htp[:, 2 * t] = h[128 * t : 128 * (t + 1)]
            htp[:, 2 * t + 1] = tl[128 * t : 128 * (t + 1)]
            rlp[:, t] = r[128 * t : 128 * (t + 1)]
        in_maps.append(
            {
                "ea": ea,
                "relx": relx,
                "rely": rely,
                "alphaf": alphaf,
                "htidx": htp,
                "relidx": rlp,
            }
        )
    return in_maps


def unshard_out(results):
    """results: list of per-core dicts with 'out' [128, NT] f32."""
    full = np.empty(BATCH, np.float32)
    for c in range(NCORES):
        o = np.asarray(results[c]["out"])          # [128, NT], col = t
        full[c * B_LOC : (c + 1) * B_LOC] = o.T.ravel()
    return full


def kernel(head_idx, relation_idx, tail_idx, entity_embedding,
           relation_embedding, alpha_embedding):
    nc = _get_program()
    in_maps = make_in_maps(head_idx, relation_idx, tail_idx, entity_embedding,
                           relation_embedding, alpha_embedding)
    res = run_bass_kernel_spmd(nc, in_maps, list(range(NCORES)))
    return unshard_out(res.results)


# revision 7
# speedup vs baseline: 1.3713x; 1.1207x over previous
"""Trainium2 Bass kernel for nn_DKSTE_85315230367936 (embedding_lookup).

Math (per batch element b, dim d, K=2 planes):
    x = sign(rel[b,d,0]); y = sign(rel[b,d,1]); a = sign(alpha[b,d])
    s = (x+y)/2 ; dd = (x-y)/2   (exactly one of s, dd is nonzero)
    term = h0*(s*t0 - dd*a*t1) + h1*(dd*t0 + s*a*t1)
    out[b] = sqrt(sum_d term^2)

Only |term| matters (squared).  With m = [sign(x)==sign(y)] and
phi = a*sign(x*y):
    m=1: term^2 = u^2,  u = h0*t0 + h1*(phi*t1)
    m=0: term^2 = v^2,  v = h1*t0 + h0*(phi*t1)
so z = select(m, u, v), out = sqrt(sum_d z^2); phi*t1 is shared.

Per-core (1024 elems, 8 tiles of 128):
  - entity table replicated per core: [200000, 1024] fp16, row [E1|E0]
    (k-planes swapped so the wide fused op below gets [h1|h0] directly).
  - device builds sda [500, 768] fp16-units: [phi(512 fp16) | m(512 int8)]
    from the uploaded (host-deinterleaved) relation x/y planes + alpha.
  - per tile, three [128,1]-offset indirect gathers (the only offset
    layout the SWDGE ucode supports; multi-column offsets fetch
    contiguous bursts instead): head row, tail row, sda row.  All 24
    calls are interleaved tile-by-tile so compute chases the gathers.
  - per tile DVE: u1,v1 into UV1=[u1|v1]; pt1=phi*t1;
    W=[h1|h0]*bcast(pt1)=[u2|v2]; UV=UV1+W=[u|v];
    copy_predicated(v-half, m, u-half) -> z; ACT square-accumulate.
"""

import sys

for _p in ("/opt/trn_rl_repo",):
    if _p not in sys.path:
        sys.path.insert(0, _p)

import numpy as np

import concourse.bass as bass
import concourse.bacc as bacc
import concourse.tile as tile
from concourse import mybir
from concourse.bass_utils import run_bass_kernel_spmd

NENTITY, NRELATION, EMB_DIM, K = 200000, 500, 512, 2
BATCH = 8192
NCORES = 8
B_LOC = BATCH // NCORES            # 1024 batch elements per core
NT = B_LOC // 128                  # 8 tiles of 128 per core
CDT = mybir.dt.float16
NP_CDT = np.float16

F32 = mybir.dt.float32
I8 = mybir.dt.int8
I16 = mybir.dt.int16
I32 = mybir.dt.int32
AF = mybir.ActivationFunctionType
ALU = mybir.AluOpType

REL_P = 125                                  # 4 relation rows per partition
RX_FREE = NRELATION * EMB_DIM // REL_P       # 2048
SDA_W = EMB_DIM + EMB_DIM // 2               # 768 fp16 units = 1536 B


def build_program():
    nc = bacc.Bacc("TRN2", target_bir_lowering=False, debug=False,
                   num_swdge_queues=4)

    ea = nc.declare_dram_parameter("ea", [NENTITY, 2 * EMB_DIM], CDT, isOutput=False)
    relx = nc.declare_dram_parameter("relx", [REL_P, RX_FREE], CDT, isOutput=False)
    rely = nc.declare_dram_parameter("rely", [REL_P, RX_FREE], CDT, isOutput=False)
    alphaf = nc.declare_dram_parameter("alphaf", [REL_P, RX_FREE], CDT, isOutput=False)
    htidx = nc.declare_dram_parameter("htidx", [128, 2 * NT], I32, isOutput=False)
    relidx = nc.declare_dram_parameter("relidx", [128, NT], I32, isOutput=False)
    out = nc.declare_dram_parameter("out", [128, NT], F32, isOutput=True)

    with tile.TileContext(nc) as tc:
        with (
            tc.tile_pool(name="dram", bufs=1, space="DRAM") as dramp,
            tc.tile_pool(name="idx", bufs=1) as idxp,
            tc.tile_pool(name="prep", bufs=1) as prep,
            tc.tile_pool(name="gat", bufs=8) as gat,
            tc.tile_pool(name="gsd", bufs=8) as gsdp,
            tc.tile_pool(name="wrk", bufs=2) as wrk,
            tc.tile_pool(name="outp", bufs=1) as outp,
        ):
            sda = dramp.tile([NRELATION, SDA_W], CDT)

            # ---- uploads.  Index tables first (gathers depend on them),
            # then the relation tables split across the two HWDGE rings so
            # the sda critical path starts as early as possible.
            ht_t = idxp.tile([128, 2 * NT], I32)
            nc.sync.dma_start(out=ht_t[:], in_=htidx[:])
            rel_t = idxp.tile([128, NT], I32)
            nc.sync.dma_start(out=rel_t[:], in_=relidx[:])
            alsb = prep.tile([REL_P, RX_FREE], CDT)
            nc.scalar.dma_start(out=alsb[:], in_=alphaf[:])
            relxb = prep.tile([REL_P, RX_FREE], CDT)
            nc.sync.dma_start(out=relxb[:], in_=relx[:])
            relyb = prep.tile([REL_P, RX_FREE], CDT)
            nc.scalar.dma_start(out=relyb[:], in_=rely[:])

            # ---- sign-table precompute --------------------------------
            asg = prep.tile([REL_P, RX_FREE], CDT)
            nc.scalar.activation(asg[:], alsb[:], AF.Sign)
            xy = prep.tile([REL_P, RX_FREE], CDT)
            nc.vector.tensor_tensor(out=xy[:], in0=relxb[:], in1=relyb[:],
                                    op=ALU.mult)
            sxy = prep.tile([REL_P, RX_FREE], CDT)
            nc.scalar.activation(sxy[:], xy[:], AF.Sign)

            # sda image: 4 relation blocks of 768 per partition,
            # each block = [phi(512 fp16) | m(512 int8)]
            sda_sb = prep.tile([REL_P, 4 * SDA_W], CDT)
            sda_sbv = sda_sb[:].rearrange("p (r c) -> p r c", c=SDA_W)
            a3 = asg[:].rearrange("p (r d) -> p r d", d=EMB_DIM)
            s3 = sxy[:].rearrange("p (r d) -> p r d", d=EMB_DIM)
            xy3 = xy[:].rearrange("p (r d) -> p r d", d=EMB_DIM)
            nc.vector.tensor_tensor(
                out=sda_sbv[:, :, 0:EMB_DIM], in0=a3, in1=s3, op=ALU.mult
            )
            nc.vector.tensor_scalar(
                out=sda_sbv[:, :, EMB_DIM:SDA_W].bitcast(I8),
                in0=xy3, scalar1=0.0, scalar2=None, op0=ALU.is_gt,
            )
            nc.sync.dma_start(
                out=sda[:].rearrange("(p r) w -> p (r w)", r=4), in_=sda_sb[:]
            )

            # ---- main loop: 3 gathers per tile, compute chases --------
            scores = outp.tile([128, NT], F32)
            junk = outp.tile([128, EMB_DIM], CDT)
            qn = [0]

            def igather(out_ap, in_ap, off_ap):
                return nc.gpsimd.indirect_dma_start(
                    out=out_ap, out_offset=None, in_=in_ap,
                    in_offset=bass.IndirectOffsetOnAxis(ap=off_ap, axis=0),
                )

            ghts, gsds = [], []
            for t in range(NT):
                ght = gat.tile([128, 2048], CDT, tag="ght")
                igather(ght[:, 0:1024], ea[:], ht_t[:, 2 * t : 2 * t + 1])
                igather(ght[:, 1024:2048], ea[:], ht_t[:, 2 * t + 1 : 2 * t + 2])
                ghts.append(ght)
            for t in range(NT):
                gsd = gsdp.tile([128, SDA_W], CDT, tag="gsd")
                igather(gsd[:], sda[:], rel_t[:, t : t + 1])
                gsds.append(gsd)

            for t in range(NT):
                ght = ghts[t]
                gsd = gsds[t]
                # entity row layout [E1|E0]: head slice = [h1|h0]
                h1 = ght[:, 0:512]
                h0 = ght[:, 512:1024]
                t1 = ght[:, 1024:1536]
                t0 = ght[:, 1536:2048]
                phi = gsd[:, 0:EMB_DIM]
                msk = gsd[:, EMB_DIM:SDA_W].bitcast(I8)

                uv1 = wrk.tile([128, 2, EMB_DIM], CDT, tag="uv1")
                nc.vector.tensor_tensor(out=uv1[:, 0, :], in0=h0, in1=t0,
                                        op=ALU.mult)
                nc.vector.tensor_tensor(out=uv1[:, 1, :], in0=h1, in1=t0,
                                        op=ALU.mult)
                pt1 = wrk.tile([128, EMB_DIM], CDT, tag="pt1")
                nc.vector.tensor_tensor(out=pt1[:], in0=phi, in1=t1,
                                        op=ALU.mult)
                w2 = wrk.tile([128, 2, EMB_DIM], CDT, tag="w2")
                pt1b = pt1[:].rearrange("p (o d) -> p o d", o=1).broadcast_to(
                    [128, 2, EMB_DIM]
                )
                nc.vector.tensor_tensor(
                    out=w2[:],
                    in0=ght[:, 0:1024].rearrange("p (o d) -> p o d", d=EMB_DIM),
                    in1=pt1b, op=ALU.mult,
                )
                uv = wrk.tile([128, 2, EMB_DIM], CDT, tag="uv")
                nc.vector.tensor_tensor(out=uv[:], in0=uv1[:], in1=w2[:],
                                        op=ALU.add)
                # uv = [u | v]; z = select(m, u, v) in place in the v half
                nc.vector.copy_predicated(uv[:, 1, :], msk, uv[:, 0, :])

                nc.scalar.activation(
                    junk[:], uv[:, 1, :], AF.Square,
                    accum_out=scores[:, t : t + 1],
                )

            res = outp.tile([128, NT], F32)
            nc.scalar.activation(res[:], scores[:], AF.Sqrt)
            nc.sync.dma_start(out=out[:], in_=res[:])

    nc.compile()
    return nc


_NC_CACHE = None


def _get_program():
    global _NC_CACHE
    if _NC_CACHE is None:
        _NC_CACHE = build_program()
    return _NC_CACHE


def make_in_maps(head_idx, relation_idx, tail_idx, entity_embedding,
                 relation_embedding, alpha_embedding):
    """Host-side sharding: slice batch 1024/core, replicate tables."""
    head_idx = np.asarray(head_idx).astype(np.int32)
    relation_idx = np.asarray(relation_idx).astype(np.int32)
    tail_idx = np.asarray(tail_idx).astype(np.int32)
    ent = np.asarray(entity_embedding)
    rel = np.asarray(relation_embedding)
    alp = np.asarray(alpha_embedding)

    # ea row r = [E[r,:,0,1] | E[r,:,0,0]]  (k planes swapped)
    ea = np.ascontiguousarray(
        ent[:, :, 0, ::-1].transpose(0, 2, 1).reshape(NENTITY, 2 * EMB_DIM)
    ).astype(NP_CDT)
    relx = np.ascontiguousarray(rel[:, :, 0]).astype(NP_CDT).reshape(REL_P, RX_FREE)
    rely = np.ascontiguousarray(rel[:, :, 1]).astype(NP_CDT).reshape(REL_P, RX_FREE)
    alphaf = alp.astype(NP_CDT).reshape(REL_P, RX_FREE)

    in_maps = []
    for c in range(NCORES):
        lo = c * B_LOC
        h = head_idx[lo : lo + B_LOC]
        tl = tail_idx[lo : lo + B_LOC]
        r = relation_idx[lo : lo + B_LOC]
        htp = np.empty((128, 2 * NT), np.int32)
        rlp = np.empty((128, NT), np.int32)
        for t in range(NT):
            htp[:, 2 * t] = h[128 * t : 128 * (t + 1)]
            htp[:, 2 * t + 1] = tl[128 * t : 128 * (t + 1)]
            rlp[:, t] = r[128 * t : 128 * (t + 1)]
        in_maps.append(
            {
                "ea": ea,
                "relx": relx,
                "rely": rely,
                "alphaf": alphaf,
                "htidx": htp,
                "relidx": rlp,
            }
        )
    return in_maps


def unshard_out(results):
    """results: list of per-core dicts with 'out' [128, NT] f32."""
    full = np.empty(BATCH, np.float32)
    for c in range(NCORES):
        o = np.asarray(results[c]["out"])          # [128, NT], col = t
        full[c * B_LOC : (c + 1) * B_LOC] = o.T.ravel()
    return full


def kernel(head_idx, relation_idx, tail_idx, entity_embedding,
           relation_embedding, alpha_embedding):
    nc = _get_program()
    in_maps = make_in_maps(head_idx, relation_idx, tail_idx, entity_embedding,
                           relation_embedding, alpha_embedding)
    res = run_bass_kernel_spmd(nc, in_maps, list(range(NCORES)))
    return unshard_out(res.results)


# revision 8
# speedup vs baseline: 1.4102x; 1.0284x over previous
"""Trainium2 Bass kernel for nn_DKSTE_85315230367936 (embedding_lookup).

Math (per batch element b, dim d, K=2 planes):
    x = sign(rel[b,d,0]); y = sign(rel[b,d,1]); a = sign(alpha[b,d])
    s = (x+y)/2 ; dd = (x-y)/2   (exactly one of s, dd is nonzero)
    term = h0*(s*t0 - dd*a*t1) + h1*(dd*t0 + s*a*t1)
    out[b] = sqrt(sum_d term^2)

Only |term| matters (squared).  With m = [sign(x)==sign(y)] and
phi = a*sign(x*y):
    m=1: term^2 = u^2,  u = h0*t0 + h1*(phi*t1)
    m=0: term^2 = v^2,  v = h1*t0 + h0*(phi*t1)
so z = select(m, u, v), out = sqrt(sum_d z^2); phi*t1 is shared.

Per-core (1024 elems, 8 tiles of 128):
  - entity table replicated per core: [200000, 1024] fp16, row [E1|E0]
    (k-planes swapped so the wide fused op below gets [h1|h0] directly).
  - device builds sda [500, 768] fp16-units: [phi(512 fp16) | m(512 int8)]
    from the uploaded (host-deinterleaved) relation x/y planes + alpha.
  - per tile, three [128,1]-offset indirect gathers (the only offset
    layout the SWDGE ucode supports; multi-column offsets fetch
    contiguous bursts instead): head row, tail row, sda row.  All 24
    calls are interleaved tile-by-tile so compute chases the gathers.
  - per tile DVE: u1,v1 into UV1=[u1|v1]; pt1=phi*t1;
    W=[h1|h0]*bcast(pt1)=[u2|v2]; UV=UV1+W=[u|v];
    copy_predicated(v-half, m, u-half) -> z; ACT square-accumulate.
"""

import sys

for _p in ("/opt/trn_rl_repo",):
    if _p not in sys.path:
        sys.path.insert(0, _p)

import numpy as np

import concourse.bass as bass
import concourse.bacc as bacc
import concourse.tile as tile
from concourse import mybir
from concourse.bass_utils import run_bass_kernel_spmd

NENTITY, NRELATION, EMB_DIM, K = 200000, 500, 512, 2
BATCH = 8192
NCORES = 8
B_LOC = BATCH // NCORES            # 1024 batch elements per core
NT = B_LOC // 128                  # 8 tiles of 128 per core
CDT = mybir.dt.float16
NP_CDT = np.float16

F32 = mybir.dt.float32
I8 = mybir.dt.int8
I16 = mybir.dt.int16
I32 = mybir.dt.int32
AF = mybir.ActivationFunctionType
ALU = mybir.AluOpType

REL_P = 125                                  # 4 relation rows per partition
RX_FREE = NRELATION * EMB_DIM // REL_P       # 2048
SDA_W = EMB_DIM + EMB_DIM // 2               # 768 fp16 units = 1536 B


def build_program():
    nc = bacc.Bacc("TRN2", target_bir_lowering=False, debug=False,
                   num_swdge_queues=4)

    ea = nc.declare_dram_parameter("ea", [NENTITY, 2 * EMB_DIM], CDT, isOutput=False)
    relx = nc.declare_dram_parameter("relx", [REL_P, RX_FREE], CDT, isOutput=False)
    rely = nc.declare_dram_parameter("rely", [REL_P, RX_FREE], CDT, isOutput=False)
    alphaf = nc.declare_dram_parameter("alphaf", [REL_P, RX_FREE], CDT, isOutput=False)
    htidx = nc.declare_dram_parameter("htidx", [128, 2 * NT], I32, isOutput=False)
    relidx = nc.declare_dram_parameter("relidx", [128, NT], I32, isOutput=False)
    out = nc.declare_dram_parameter("out", [128, NT], F32, isOutput=True)

    with tile.TileContext(nc) as tc:
        with (
            tc.tile_pool(name="dram", bufs=1, space="DRAM") as dramp,
            tc.tile_pool(name="idx", bufs=1) as idxp,
            tc.tile_pool(name="prep", bufs=1) as prep,
            tc.tile_pool(name="gat", bufs=8) as gat,
            tc.tile_pool(name="gsd", bufs=8) as gsdp,
            tc.tile_pool(name="wrk", bufs=2) as wrk,
            tc.tile_pool(name="outp", bufs=1) as outp,
        ):
            sda = dramp.tile([NRELATION, SDA_W], CDT)

            # ---- uploads.  The relation tables go FIRST on both HWDGE
            # rings and the index tables AFTER them on the same rings:
            # rings are FIFO, so the gathers (which need the indices)
            # cannot start flooding the SDMA engines until the small
            # relation tables have landed.  Otherwise the 4MB of entity
            # gather payload starves the 2.5MB of uploads and the whole
            # sda critical path slips by ~20us.
            alsb = prep.tile([REL_P, RX_FREE], CDT)
            nc.scalar.dma_start(out=alsb[:], in_=alphaf[:])
            relxb = prep.tile([REL_P, RX_FREE], CDT)
            nc.sync.dma_start(out=relxb[:], in_=relx[:])
            relyb = prep.tile([REL_P, RX_FREE], CDT)
            nc.scalar.dma_start(out=relyb[:], in_=rely[:])
            ht_t = idxp.tile([128, 2 * NT], I32)
            nc.sync.dma_start(out=ht_t[:], in_=htidx[:])
            rel_t = idxp.tile([128, NT], I32)
            nc.scalar.dma_start(out=rel_t[:], in_=relidx[:])

            # ---- sign-table precompute --------------------------------
            asg = prep.tile([REL_P, RX_FREE], CDT)
            nc.scalar.activation(asg[:], alsb[:], AF.Sign)
            xy = prep.tile([REL_P, RX_FREE], CDT)
            nc.vector.tensor_tensor(out=xy[:], in0=relxb[:], in1=relyb[:],
                                    op=ALU.mult)
            sxy = prep.tile([REL_P, RX_FREE], CDT)
            nc.scalar.activation(sxy[:], xy[:], AF.Sign)

            # sda image: 4 relation blocks of 768 per partition,
            # each block = [phi(512 fp16) | m(512 int8)]
            sda_sb = prep.tile([REL_P, 4 * SDA_W], CDT)
            sda_sbv = sda_sb[:].rearrange("p (r c) -> p r c", c=SDA_W)
            a3 = asg[:].rearrange("p (r d) -> p r d", d=EMB_DIM)
            s3 = sxy[:].rearrange("p (r d) -> p r d", d=EMB_DIM)
            xy3 = xy[:].rearrange("p (r d) -> p r d", d=EMB_DIM)
            nc.vector.tensor_tensor(
                out=sda_sbv[:, :, 0:EMB_DIM], in0=a3, in1=s3, op=ALU.mult
            )
            nc.vector.tensor_scalar(
                out=sda_sbv[:, :, EMB_DIM:SDA_W].bitcast(I8),
                in0=xy3, scalar1=0.0, scalar2=None, op0=ALU.is_gt,
            )
            nc.sync.dma_start(
                out=sda[:].rearrange("(p r) w -> p (r w)", r=4), in_=sda_sb[:]
            )

            # ---- main loop: 3 gathers per tile, compute chases --------
            scores = outp.tile([128, NT], F32)
            junk = outp.tile([128, EMB_DIM], CDT)
            qn = [0]

            def igather(out_ap, in_ap, off_ap):
                return nc.gpsimd.indirect_dma_start(
                    out=out_ap, out_offset=None, in_=in_ap,
                    in_offset=bass.IndirectOffsetOnAxis(ap=off_ap, axis=0),
                )

            ghts, gsds = [], []
            for t in range(NT):
                ght = gat.tile([128, 2048], CDT, tag="ght")
                igather(ght[:, 0:1024], ea[:], ht_t[:, 2 * t : 2 * t + 1])
                igather(ght[:, 1024:2048], ea[:], ht_t[:, 2 * t + 1 : 2 * t + 2])
                ghts.append(ght)
            for t in range(NT):
                gsd = gsdp.tile([128, SDA_W], CDT, tag="gsd")
                igather(gsd[:], sda[:], rel_t[:, t : t + 1])
                gsds.append(gsd)

            for t in range(NT):
                ght = ghts[t]
                gsd = gsds[t]
                # entity row layout [E1|E0]: head slice = [h1|h0]
                h1 = ght[:, 0:512]
                h0 = ght[:, 512:1024]
                t1 = ght[:, 1024:1536]
                t0 = ght[:, 1536:2048]
                phi = gsd[:, 0:EMB_DIM]
                msk = gsd[:, EMB_DIM:SDA_W].bitcast(I8)

                uv1 = wrk.tile([128, 2, EMB_DIM], CDT, tag="uv1")
                nc.vector.tensor_tensor(out=uv1[:, 0, :], in0=h0, in1=t0,
                                        op=ALU.mult)
                nc.vector.tensor_tensor(out=uv1[:, 1, :], in0=h1, in1=t0,
                                        op=ALU.mult)
                pt1 = wrk.tile([128, EMB_DIM], CDT, tag="pt1")
                nc.vector.tensor_tensor(out=pt1[:], in0=phi, in1=t1,
                                        op=ALU.mult)
                w2 = wrk.tile([128, 2, EMB_DIM], CDT, tag="w2")
                pt1b = pt1[:].rearrange("p (o d) -> p o d", o=1).broadcast_to(
                    [128, 2, EMB_DIM]
                )
                nc.vector.tensor_tensor(
                    out=w2[:],
                    in0=ght[:, 0:1024].rearrange("p (o d) -> p o d", d=EMB_DIM),
                    in1=pt1b, op=ALU.mult,
                )
                uv = wrk.tile([128, 2, EMB_DIM], CDT, tag="uv")
                nc.vector.tensor_tensor(out=uv[:], in0=uv1[:], in1=w2[:],
                                        op=ALU.add)
                # uv = [u | v]; z = select(m, u, v) in place in the v half
                nc.vector.copy_predicated(uv[:, 1, :], msk, uv[:, 0, :])

                nc.scalar.activation(
                    junk[:], uv[:, 1, :], AF.Square,
                    accum_out=scores[:, t : t + 1],
                )

            res = outp.tile([128, NT], F32)
            nc.scalar.activation(res[:], scores[:], AF.Sqrt)
            nc.sync.dma_start(out=out[:], in_=res[:])

    nc.compile()
    return nc


_NC_CACHE = None


def _get_program():
    global _NC_CACHE
    if _NC_CACHE is None:
        _NC_CACHE = build_program()
    return _NC_CACHE


def make_in_maps(head_idx, relation_idx, tail_idx, entity_embedding,
                 relation_embedding, alpha_embedding):
    """Host-side sharding: slice batch 1024/core, replicate tables."""
    head_idx = np.asarray(head_idx).astype(np.int32)
    relation_idx = np.asarray(relation_idx).astype(np.int32)
    tail_idx = np.asarray(tail_idx).astype(np.int32)
    ent = np.asarray(entity_embedding)
    rel = np.asarray(relation_embedding)
    alp = np.asarray(alpha_embedding)

    # ea row r = [E[r,:,0,1] | E[r,:,0,0]]  (k planes swapped)
    ea = np.ascontiguousarray(
        ent[:, :, 0, ::-1].transpose(0, 2, 1).reshape(NENTITY, 2 * EMB_DIM)
    ).astype(NP_CDT)
    relx = np.ascontiguousarray(rel[:, :, 0]).astype(NP_CDT).reshape(REL_P, RX_FREE)
    rely = np.ascontiguousarray(rel[:, :, 1]).astype(NP_CDT).reshape(REL_P, RX_FREE)
    alphaf = alp.astype(NP_CDT).reshape(REL_P, RX_FREE)

    in_maps = []
    for c in range(NCORES):
        lo = c * B_LOC
        h = head_idx[lo : lo + B_LOC]
        tl = tail_idx[lo : lo + B_LOC]
        r = relation_idx[lo : lo + B_LOC]
        htp = np.empty((128, 2 * NT), np.int32)
        rlp = np.empty((128, NT), np.int32)
        for t in range(NT):
            htp[:, 2 * t] = h[128 * t : 128 * (t + 1)]
            htp[:, 2 * t + 1] = tl[128 * t : 128 * (t + 1)]
            rlp[:, t] = r[128 * t : 128 * (t + 1)]
        in_maps.append(
            {
                "ea": ea,
                "relx": relx,
                "rely": rely,
                "alphaf": alphaf,
                "htidx": htp,
                "relidx": rlp,
            }
        )
    return in_maps


def unshard_out(results):
    """results: list of per-core dicts with 'out' [128, NT] f32."""
    full = np.empty(BATCH, np.float32)
    for c in range(NCORES):
        o = np.asarray(results[c]["out"])          # [128, NT], col = t
        full[c * B_LOC : (c + 1) * B_LOC] = o.T.ravel()
    return full


def kernel(head_idx, relation_idx, tail_idx, entity_embedding,
           relation_embedding, alpha_embedding):
    nc = _get_program()
    in_maps = make_in_maps(head_idx, relation_idx, tail_idx, entity_embedding,
                           relation_embedding, alpha_embedding)
    res = run_bass_kernel_spmd(nc, in_maps, list(range(NCORES)))
    return unshard_out(res.results)


# revision 12
# speedup vs baseline: 1.4680x; 1.0410x over previous
"""Trainium2 Bass kernel for nn_DKSTE_85315230367936 (embedding_lookup).

Math (per batch element b, dim d, K=2 planes):
    x = sign(rel[b,d,0]); y = sign(rel[b,d,1]); a = sign(alpha[b,d])
    s = (x+y)/2 ; dd = (x-y)/2   (exactly one of s, dd is nonzero)
    term = h0*(s*t0 - dd*a*t1) + h1*(dd*t0 + s*a*t1)
    out[b] = sqrt(sum_d term^2)

Only |term| matters (squared).  With m = [sign(x)==sign(y)] and
phi = a*sign(x*y):
    m=1: term^2 = u^2,  u = h0*t0 + h1*(phi*t1)
    m=0: term^2 = v^2,  v = h1*t0 + h0*(phi*t1)
so z = select(m, u, v), out = sqrt(sum_d z^2); phi*t1 is shared.

Per-core (1024 elems, 8 tiles of 128):
  - entity table replicated per core: [200000, 1024] fp16, row [E1|E0]
    (k-planes swapped so the wide fused op below gets [h1|h0] directly).
  - device builds sda [500, 768] fp16-units: [phi(512 fp16) | m(512 int8)]
    from the uploaded (host-deinterleaved) relation x/y planes + alpha.
  - per tile, three [128,1]-offset indirect gathers (the only offset
    layout the SWDGE ucode supports; multi-column offsets fetch
    contiguous bursts instead): head row, tail row, sda row.  All 24
    calls are interleaved tile-by-tile so compute chases the gathers.
  - per tile DVE: u1,v1 into UV1=[u1|v1]; pt1=phi*t1;
    W=[h1|h0]*bcast(pt1)=[u2|v2]; UV=UV1+W=[u|v];
    copy_predicated(v-half, m, u-half) -> z; ACT square-accumulate.
"""

import sys

for _p in ("/opt/trn_rl_repo",):
    if _p not in sys.path:
        sys.path.insert(0, _p)

import numpy as np

import concourse.bass as bass
import concourse.bacc as bacc
import concourse.tile as tile
from concourse import mybir
from concourse.bass_utils import run_bass_kernel_spmd

NENTITY, NRELATION, EMB_DIM, K = 200000, 500, 512, 2
BATCH = 8192
NCORES = 8
B_LOC = BATCH // NCORES            # 1024 batch elements per core
NT = B_LOC // 128                  # 8 tiles of 128 per core
CDT = mybir.dt.float16
NP_CDT = np.float16

F32 = mybir.dt.float32
F8 = mybir.dt.float8e4
I8 = mybir.dt.int8
I16 = mybir.dt.int16
I32 = mybir.dt.int32
AF = mybir.ActivationFunctionType
ALU = mybir.AluOpType

REL_P = 125                                  # 4 relation rows per partition
RX_FREE = NRELATION * EMB_DIM // REL_P       # 2048
SDA_W = EMB_DIM + EMB_DIM // 2               # 768 fp16 units = 1536 B


def build_program():
    nc = bacc.Bacc("TRN2", target_bir_lowering=False, debug=False,
                   num_swdge_queues=4)

    ea = nc.declare_dram_parameter("ea", [NENTITY, 2 * EMB_DIM], CDT, isOutput=False)
    relx = nc.declare_dram_parameter("relx", [REL_P, RX_FREE], F8, isOutput=False)
    rely = nc.declare_dram_parameter("rely", [REL_P, RX_FREE], F8, isOutput=False)
    alphaf = nc.declare_dram_parameter("alphaf", [REL_P, RX_FREE], F8, isOutput=False)
    htidx = nc.declare_dram_parameter("htidx", [128, 2 * NT], I32, isOutput=False)
    relidx = nc.declare_dram_parameter("relidx", [128, NT], I32, isOutput=False)
    out = nc.declare_dram_parameter("out", [128, NT], F32, isOutput=True)

    with tile.TileContext(nc) as tc:
        with (
            tc.tile_pool(name="dram", bufs=1, space="DRAM") as dramp,
            tc.tile_pool(name="idx", bufs=1) as idxp,
            tc.tile_pool(name="prep", bufs=1) as prep,
            tc.tile_pool(name="gat", bufs=8) as gat,
            tc.tile_pool(name="gsd", bufs=8) as gsdp,
            tc.tile_pool(name="wrk", bufs=2) as wrk,
            tc.tile_pool(name="outp", bufs=1) as outp,
        ):
            sda = dramp.tile([NRELATION, SDA_W], CDT)

            # ---- uploads.  The relation tables go FIRST on both HWDGE
            # rings and the index tables AFTER them on the same rings:
            # rings are FIFO, so the gathers (which need the indices)
            # cannot start flooding the SDMA engines until the small
            # relation tables have landed.  Otherwise the 4MB of entity
            # gather payload starves the 2.5MB of uploads and the whole
            # sda critical path slips by ~20us.
            alsb = prep.tile([REL_P, RX_FREE], F8)
            nc.scalar.dma_start(out=alsb[:], in_=alphaf[:])
            relxb = prep.tile([REL_P, RX_FREE], F8)
            nc.sync.dma_start(out=relxb[:], in_=relx[:])
            relyb = prep.tile([REL_P, RX_FREE], F8)
            nc.scalar.dma_start(out=relyb[:], in_=rely[:])
            ht_t = idxp.tile([128, 2 * NT], I32)
            nc.sync.dma_start(out=ht_t[:], in_=htidx[:])
            rel_t = idxp.tile([128, NT], I32)
            nc.scalar.dma_start(out=rel_t[:], in_=relidx[:])

            # ---- sign-table precompute --------------------------------
            asg = prep.tile([REL_P, RX_FREE], CDT)
            nc.scalar.activation(asg[:], alsb[:], AF.Sign)
            xy = prep.tile([REL_P, RX_FREE], CDT)
            nc.vector.tensor_tensor(out=xy[:], in0=relxb[:], in1=relyb[:],
                                    op=ALU.mult)
            sxy = prep.tile([REL_P, RX_FREE], CDT)
            nc.scalar.activation(sxy[:], xy[:], AF.Sign)

            # sda image: 4 relation blocks of 768 per partition,
            # each block = [phi(512 fp16) | m(512 int8)]
            sda_sb = prep.tile([REL_P, 4 * SDA_W], CDT)
            sda_sbv = sda_sb[:].rearrange("p (r c) -> p r c", c=SDA_W)
            a3 = asg[:].rearrange("p (r d) -> p r d", d=EMB_DIM)
            s3 = sxy[:].rearrange("p (r d) -> p r d", d=EMB_DIM)
            xy3 = xy[:].rearrange("p (r d) -> p r d", d=EMB_DIM)
            nc.vector.tensor_tensor(
                out=sda_sbv[:, :, 0:EMB_DIM], in0=a3, in1=s3, op=ALU.mult
            )
            nc.vector.tensor_scalar(
                out=sda_sbv[:, :, EMB_DIM:SDA_W].bitcast(I8),
                in0=xy3, scalar1=0.0, scalar2=None, op0=ALU.is_gt,
            )
            nc.sync.dma_start(
                out=sda[:].rearrange("(p r) w -> p (r w)", r=4), in_=sda_sb[:]
            )

            # ---- main loop: 3 gathers per tile, compute chases --------
            scores = outp.tile([128, NT], F32)
            junk = outp.tile([128, EMB_DIM], CDT)
            qn = [0]

            def igather(out_ap, in_ap, off_ap):
                return nc.gpsimd.indirect_dma_start(
                    out=out_ap, out_offset=None, in_=in_ap,
                    in_offset=bass.IndirectOffsetOnAxis(ap=off_ap, axis=0),
                )

            ghts, gsds = [], []
            for t in range(NT):
                ght = gat.tile([128, 2048], CDT, tag="ght")
                igather(ght[:, 0:1024], ea[:], ht_t[:, 2 * t : 2 * t + 1])
                igather(ght[:, 1024:2048], ea[:], ht_t[:, 2 * t + 1 : 2 * t + 2])
                ghts.append(ght)
            for t in range(NT):
                gsd = gsdp.tile([128, SDA_W], CDT, tag="gsd")
                igather(gsd[:], sda[:], rel_t[:, t : t + 1])
                gsds.append(gsd)

            for t in range(NT):
                ght = ghts[t]
                gsd = gsds[t]
                # entity row layout [E1|E0]: head slice = [h1|h0]
                h1 = ght[:, 0:512]
                h0 = ght[:, 512:1024]
                t1 = ght[:, 1024:1536]
                t0 = ght[:, 1536:2048]
                phi = gsd[:, 0:EMB_DIM]
                msk = gsd[:, EMB_DIM:SDA_W].bitcast(I8)

                uv1 = wrk.tile([128, 2, EMB_DIM], CDT, tag="uv1")
                nc.vector.tensor_tensor(out=uv1[:, 0, :], in0=h0, in1=t0,
                                        op=ALU.mult)
                nc.vector.tensor_tensor(out=uv1[:, 1, :], in0=h1, in1=t0,
                                        op=ALU.mult)
                pt1 = wrk.tile([128, EMB_DIM], CDT, tag="pt1")
                nc.vector.tensor_tensor(out=pt1[:], in0=phi, in1=t1,
                                        op=ALU.mult)
                w2 = wrk.tile([128, 2, EMB_DIM], CDT, tag="w2")
                pt1b = pt1[:].rearrange("p (o d) -> p o d", o=1).broadcast_to(
                    [128, 2, EMB_DIM]
                )
                nc.vector.tensor_tensor(
                    out=w2[:],
                    in0=ght[:, 0:1024].rearrange("p (o d) -> p o d", d=EMB_DIM),
                    in1=pt1b, op=ALU.mult,
                )
                uv = wrk.tile([128, 2, EMB_DIM], CDT, tag="uv")
                nc.vector.tensor_tensor(out=uv[:], in0=uv1[:], in1=w2[:],
                                        op=ALU.add)
                # uv = [u | v]; z = select(m, u, v) in place in the v half
                nc.vector.copy_predicated(uv[:, 1, :], msk, uv[:, 0, :])

                nc.scalar.activation(
                    junk[:], uv[:, 1, :], AF.Square,
                    accum_out=scores[:, t : t + 1],
                )

            res = outp.tile([128, NT], F32)
            nc.scalar.activation(res[:], scores[:], AF.Sqrt)
            nc.sync.dma_start(out=out[:], in_=res[:])

    nc.compile()
    return nc


_NC_CACHE = None


def _get_program():
    global _NC_CACHE
    if _NC_CACHE is None:
        _NC_CACHE = build_program()
    return _NC_CACHE


def make_in_maps(head_idx, relation_idx, tail_idx, entity_embedding,
                 relation_embedding, alpha_embedding):
    """Host-side sharding: slice batch 1024/core, replicate tables."""
    head_idx = np.asarray(head_idx).astype(np.int32)
    relation_idx = np.asarray(relation_idx).astype(np.int32)
    tail_idx = np.asarray(tail_idx).astype(np.int32)
    ent = np.asarray(entity_embedding)
    rel = np.asarray(relation_embedding)
    alp = np.asarray(alpha_embedding)

    # ea row r = [E[r,:,0,1] | E[r,:,0,0]]  (k planes swapped)
    ea = np.ascontiguousarray(
        ent[:, :, 0, ::-1].transpose(0, 2, 1).reshape(NENTITY, 2 * EMB_DIM)
    ).astype(NP_CDT)
    import ml_dtypes
    NP_F8 = ml_dtypes.float8_e4m3  # mybir float8e4 flavor (max 240)
    # only signs of x/y/alpha are consumed on device; scale by 2^7 before
    # the fp8 cast so tiny values don't flush to zero and lose their sign
    relx = np.ascontiguousarray(rel[:, :, 0] * 128.0).astype(NP_F8).reshape(REL_P, RX_FREE)
    rely = np.ascontiguousarray(rel[:, :, 1] * 128.0).astype(NP_F8).reshape(REL_P, RX_FREE)
    alphaf = (alp * 128.0).astype(NP_F8).reshape(REL_P, RX_FREE)

    in_maps = []
    for c in range(NCORES):
        lo = c * B_LOC
        h = head_idx[lo : lo + B_LOC]
        tl = tail_idx[lo : lo + B_LOC]
        r = relation_idx[lo : lo + B_LOC]
        htp = np.empty((128, 2 * NT), np.int32)
        rlp = np.empty((128, NT), np.int32)
        for t in range(NT):
            htp[:, 2 * t] = h[128 * t : 128 * (t + 1)]
            htp[:, 2 * t + 1] = tl[128 * t : 128 * (t + 1)]
            rlp[:, t] = r[128 * t : 128 * (t + 1)]
        in_maps.append(
            {
                "ea": ea,
                "relx": relx,
                "rely": rely,
                "alphaf": alphaf,
                "htidx": htp,
                "relidx": rlp,
            }
        )
    return in_maps


def unshard_out(results):
    """results: list of per-core dicts with 'out' [128, NT] f32."""
    full = np.empty(BATCH, np.float32)
    for c in range(NCORES):
        o = np.asarray(results[c]["out"])          # [128, NT], col = t
        full[c * B_LOC : (c + 1) * B_LOC] = o.T.ravel()
    return full


def kernel(head_idx, relation_idx, tail_idx, entity_embedding,
           relation_embedding, alpha_embedding):
    nc = _get_program()
    in_maps = make_in_maps(head_idx, relation_idx, tail_idx, entity_embedding,
                           relation_embedding, alpha_embedding)
    res = run_bass_kernel_spmd(nc, in_maps, list(range(NCORES)))
    return unshard_out(res.results)
